# revision 2
# baseline (speedup 1.0000x reference)
"""v7: fp16 matmul operands everywhere (PSUM stays fp32), XCH=512,
feeder-based injection of proj(etp+1)/o-proj matmuls into the attention
inner loop so the PE stays busy while ScalarE runs exp. o-copies on DVE.
Output fp16, summed in fp32 on host.
"""

import os
import sys

for _p in ("/opt/trn_rl_repo", "/root/.axon_site/_ro/trn_rl_repo"):
    if os.path.isdir(_p) and _p not in sys.path:
        sys.path.insert(0, _p)

import contextlib

import numpy as np

import concourse.bass as bass
import concourse.tile as tile
from concourse import bacc, mybir
from concourse.bass_utils import run_bass_kernel_spmd

P = 128
L = 2048
D = 1536
HL = 6
HD = 64
EQ = 384
NQK = 768
DC = D // P      # 12
LT = L // P      # 16
ACH = 512        # attention lq chunk
XCH = 512        # qkv l chunk
NCH = L // XCH   # 4
F32 = mybir.dt.float32
F16 = mybir.dt.float16
AF = mybir.ActivationFunctionType


def build_bass(repeat=1):
    nc = bacc.Bacc("TRN2", target_bir_lowering=False, debug=False, num_devices=8)
    xT = nc.dram_tensor("xT", [D, L], F16, kind="ExternalInput")
    wqkT = nc.dram_tensor("wqkT", [D, NQK], F16, kind="ExternalInput")
    wvT = nc.dram_tensor("wvT", [D, EQ], F16, kind="ExternalInput")
    woT = nc.dram_tensor("woT", [EQ, D], F16, kind="ExternalInput")
    cos2 = nc.dram_tensor("cos2", [P, L], F16, kind="ExternalInput")
    ss2 = nc.dram_tensor("ss2", [P, L], F16, kind="ExternalInput")
    out = nc.dram_tensor("out", [L, D], F16, kind="ExternalOutput")

    xT_r = xT.rearrange("(dc p) l -> p dc l", p=P)
    wqkT_r = wqkT.rearrange("(dc p) e -> p dc e", p=P)
    wvT_r = wvT.rearrange("(dc p) e -> p dc e", p=P)
    woT_r = woT.rearrange("(ec p) d -> p ec d", p=P)

    with tile.TileContext(nc) as tc:
        rep_cm = tc.For_i(0, repeat, 1) if repeat > 1 else contextlib.nullcontext()
        with rep_cm, tc.tile_pool(name="persist", bufs=1) as persist:
            qT0 = persist.tile([P, L], F16)
            qT1 = persist.tile([P, L], F16)
            qT2 = persist.tile([P, L], F16)
            kT0 = persist.tile([P, L], F16)
            kT1 = persist.tile([P, L], F16)
            kT2 = persist.tile([P, L], F16)
            oT0 = persist.tile([P, L], F16)
            oT1 = persist.tile([P, L], F16)
            oT2 = persist.tile([P, L], F16)
            qTs = (qT0, qT1, qT2)
            kTs = (kT0, kT1, kT2)
            oTs = (oT0, oT1, oT2)
            v1 = persist.tile([P, LT, HL, HD + 1], F16)
            cos_sb = persist.tile([P, L], F16)
            ss_sb = persist.tile([P, L], F16)
            ones_c = nc.const_aps.tensor(1.0, (P, 1), F32)
            nc.vector.tensor_copy(
                v1[:, :, :, HD : HD + 1], ones_c.to_broadcast([P, LT, HL, 1])
            )
            # off the critical first-weights path: issue from idle engines
            nc.scalar.dma_start(cos_sb[:], cos2[:])
            nc.scalar.dma_start(ss_sb[:], ss2[:])

            with (
                tc.tile_pool(name="s2w", bufs=2) as s2w,
                tc.tile_pool(name="s2x", bufs=2) as s2x,
                tc.tile_pool(name="s2t", bufs=2) as s2t,
                tc.tile_pool(name="s2att", bufs=4) as s2att,
                tc.tile_pool(name="s2o", bufs=3) as s2o,
                tc.tile_pool(name="s2nrm", bufs=3) as s2nrm,
                tc.tile_pool(name="ps_acc", bufs=2, space=bass.MemorySpace.PSUM) as ps_acc,
                tc.tile_pool(name="ps_s", bufs=2, space=bass.MemorySpace.PSUM) as ps_s,
                tc.tile_pool(name="ps_av", bufs=2, space=bass.MemorySpace.PSUM) as ps_av,
            ):
                def load_wqk(etp):
                    # host lays wqkT out as [q0|k0|q1|k1|q2|k2] blocks of 128
                    wqks = s2w.tile([P, DC, 2 * P], F16, tag="w")
                    nc.sync.dma_start(
                        wqks[:], wqkT_r[:, :, etp * 2 * P : (etp + 1) * 2 * P]
                    )
                    return wqks

                def rope_store(ps, etp, half, sl):
                    # stage psum->sbuf fp16 on Pool so tcos/add are all-SBUF
                    # fp16 (2x/4x DVE modes); trot reads PSUM (partition
                    # shifts are only legal from PSUM)
                    dst = (qTs if half == 0 else kTs)[etp][:, sl]
                    tcos = s2t.tile([P, XCH], F16, tag="tcos")
                    trot = s2t.tile([P, XCH], F16, tag="trot")
                    nc.vector.tensor_mul(tcos[:], ps[:, 0:XCH], cos_sb[:, sl])
                    for q_ in range(4):
                        s = (q_ ^ 1) * 32
                        d_ = q_ * 32
                        nc.vector.tensor_mul(
                            trot[d_ : d_ + 32, :],
                            ps[s : s + 32, 0:XCH],
                            ss_sb[d_ : d_ + 32, sl],
                        )
                    nc.vector.tensor_add(dst, tcos[:], trot[:])

                def proj_feeder(etp, wqks):
                    # qk projection for head-pair etp, one yield ~= one matmul
                    xcs = {}
                    xcs[0] = s2x.tile([P, DC, XCH], F16, tag="x", name="xcf")
                    nc.sync.dma_start(xcs[0][:], xT_r[:, :, 0:XCH])
                    yield
                    for c in range(NCH):
                        if c + 1 < NCH:
                            xcs[c + 1] = s2x.tile(
                                [P, DC, XCH], F16, tag="x", name="xcf"
                            )
                            nc.sync.dma_start(
                                xcs[c + 1][:],
                                xT_r[:, :, (c + 1) * XCH : (c + 2) * XCH],
                            )
                        sl = slice(c * XCH, (c + 1) * XCH)
                        xc = xcs.pop(c)
                        for half in range(2):
                            ps = ps_acc.tile([P, ACH], F32, tag="acc")
                            for dc in range(DC):
                                nc.tensor.matmul(
                                    ps[:, 0:XCH],
                                    wqks[:, dc, half * P : (half + 1) * P],
                                    xc[:, dc, :],
                                    start=(dc == 0),
                                    stop=(dc == DC - 1),
                                )
                                yield
                            rope_store(ps, etp, half, sl)
                            yield

                def o_feeder(cq):
                    # o-projection for one lq chunk; reads outT all etps.
                    # cq3 runs in the tail where ScalarE is idle -> copies on
                    # ACT there; out DMAs issue from the idle Pool queue
                    for lt in range(ACH // P):
                        l0 = cq * ACH + lt * P
                        for dn in range(D // ACH):
                            pso = ps_acc.tile([P, ACH], F32, tag="acc")
                            for ec in range(3):
                                nc.tensor.matmul(
                                    pso[:],
                                    oTs[ec][:, l0 : l0 + P],
                                    wo_sb[:, ec, dn * ACH : (dn + 1) * ACH],
                                    start=(ec == 0),
                                    stop=(ec == 2),
                                )
                                yield
                            ot = s2o.tile([P, ACH], F16, tag="o")
                            if cq == 3:
                                nc.scalar.copy(ot[:], pso[:])
                            else:
                                nc.vector.tensor_copy(ot[:], pso[:])
                            nc.sync.dma_start(
                                out[l0 : l0 + P, dn * ACH : (dn + 1) * ACH], ot[:]
                            )
                            yield

                feeders = []

                def pump(n):
                    while n > 0 and feeders:
                        try:
                            next(feeders[0])
                            n -= 1
                        except StopIteration:
                            feeders.pop(0)

                def pump_all():
                    while feeders:
                        try:
                            next(feeders[0])
                        except StopIteration:
                            feeders.pop(0)

                # ---- warmup: proj0 (qk for pair 0) + V for all 6 heads
                # first dc-blocks of wqk+x land first so matmul 0 starts ASAP
                wqks0 = s2w.tile([P, DC, 2 * P], F16, tag="w")
                xcs0 = s2x.tile([P, DC, XCH], F16, tag="x")
                nc.sync.dma_start(wqks0[:, 0:3, :], wqkT_r[:, 0:3, 0 : 2 * P])
                nc.sync.dma_start(xcs0[:, 0:3, :], xT_r[:, 0:3, 0:XCH])
                nc.sync.dma_start(wqks0[:, 3:12, :], wqkT_r[:, 3:12, 0 : 2 * P])
                for d0 in range(3, DC, 3):
                    nc.sync.dma_start(
                        xcs0[:, d0 : d0 + 3, :], xT_r[:, d0 : d0 + 3, 0:XCH]
                    )
                wv_sb = s2w.tile([P, DC, EQ], F16, tag="w")
                for d0 in range(0, DC, 6):
                    nc.sync.dma_start(
                        wv_sb[:, d0 : d0 + 6, :], wvT_r[:, d0 : d0 + 6, :]
                    )
                xc_cur = xcs0
                for c in range(NCH):
                    if c + 1 < NCH:
                        xc_next = s2x.tile([P, DC, XCH], F16, tag="x")
                        nc.sync.dma_start(
                            xc_next[:], xT_r[:, :, (c + 1) * XCH : (c + 2) * XCH]
                        )
                    sl = slice(c * XCH, (c + 1) * XCH)
                    for half in range(2):
                        ps = ps_acc.tile([P, ACH], F32, tag="acc")
                        for dc in range(DC):
                            nc.tensor.matmul(
                                ps[:, 0:XCH],
                                wqks0[:, dc, half * P : (half + 1) * P],
                                xc_cur[:, dc, :],
                                start=(dc == 0),
                                stop=(dc == DC - 1),
                            )
                        rope_store(ps, 0, half, sl)
                    # V projection for this chunk's token tiles; psum from
                    # ps_s (idle during warmup) so v never waits on rope
                    # freeing ps_acc banks
                    for lt2 in range(XCH // P):
                        lk = c * (XCH // P) + lt2
                        pvt = ps_s.tile([P, 2 * ACH], F32, name="pvt", tag="s")
                        pv = pvt[:, 0:ACH]
                        for dc in range(DC):
                            nc.tensor.matmul(
                                pv[:, 0:EQ],
                                xc_cur[:, dc, lt2 * P : (lt2 + 1) * P],
                                wv_sb[:, dc, :],
                                start=(dc == 0),
                                stop=(dc == DC - 1),
                            )
                        nc.scalar.copy(
                            v1[:, lk, :, 0:HD],
                            pv[:, 0:EQ].rearrange("p (h d) -> p h d", h=HL),
                        )
                    if c + 1 < NCH:
                        xc_cur = xc_next

                # ---- attention, with injected proj/o work
                for etp in range(3):
                    if etp < 2:
                        feeders.append(proj_feeder(etp + 1, load_wqk(etp + 1)))
                    if etp == 1:
                        wo_sb = s2w.tile([P, 3, D], F16, tag="w")
                        nc.sync.dma_start(wo_sb[:], woT_r[:])
                    for cq in range(L // ACH):
                        cqs = slice(cq * ACH, (cq + 1) * ACH)
                        pav0 = ps_av.tile([HD + 1, ACH], F32, tag="av")
                        pav1 = ps_av.tile([HD + 1, ACH], F32, tag="av")
                        for lk in range(LT):
                            pscore = ps_s.tile([P, 2 * ACH], F32, tag="s")
                            att = s2att.tile([P, 2 * ACH], F16)
                            for hh in range(2):
                                po = hh * HD
                                nc.tensor.matmul(
                                    pscore[:, hh * ACH : (hh + 1) * ACH],
                                    kTs[etp][po : po + HD, lk * P : (lk + 1) * P],
                                    qTs[etp][po : po + HD, cqs],
                                    start=True,
                                    stop=True,
                                )
                            nc.scalar.activation(att[:], pscore[:], AF.Exp, scale=0.125)
                            pump(1 if etp < 2 else 4)
                            for hh, pav in ((0, pav0), (1, pav1)):
                                nc.tensor.matmul(
                                    pav[:],
                                    v1[:, lk, 2 * etp + hh, :],
                                    att[:, hh * ACH : (hh + 1) * ACH],
                                    start=(lk == 0),
                                    stop=(lk == LT - 1),
                                )
                        for hh, pav in ((0, pav0), (1, pav1)):
                            po = hh * HD
                            dcp = s2nrm.tile([1, ACH], F32, tag="dcp")
                            nc.vector.tensor_copy(dcp[:], pav[HD : HD + 1, :])
                            rcp = s2nrm.tile([1, ACH], F32, tag="rcp")
                            nc.vector.reciprocal_approx_fast(out=rcp[:], in_=dcp[:])
                            rb = s2nrm.tile([HD, ACH], F32, tag="rb")
                            nc.gpsimd.partition_broadcast(rb[:], rcp[:], channels=HD)
                            nc.vector.tensor_mul(
                                oTs[etp][po : po + HD, cqs], pav[0:HD, :], rb[:]
                            )
                        if etp == 2:
                            feeders.append(o_feeder(cq))
                    pump_all()

    nc.compile()
    return nc


_NC_CACHE = None


def _get_nc():
    global _NC_CACHE
    if _NC_CACHE is None:
        _NC_CACHE = build_bass()
    return _NC_CACHE


def make_in_maps(x, w_qkv, w_o, cos, sin):
    x = np.asarray(x, dtype=np.float32)
    w_qkv = np.asarray(w_qkv, dtype=np.float32)
    w_o = np.asarray(w_o, dtype=np.float32)
    cos = np.asarray(cos, dtype=np.float32)
    sin = np.asarray(sin, dtype=np.float32)

    cosT = np.ascontiguousarray(cos.T)
    sinT = sin.T
    ss = np.concatenate([-sinT[0:32], sinT[32:64]], axis=0)
    cos2 = np.ascontiguousarray(np.tile(cosT, (2, 1))).astype(np.float16)
    ss2 = np.ascontiguousarray(np.tile(ss, (2, 1))).astype(np.float16)

    in_maps = []
    for c in range(8):
        b, g = c // 4, c % 4
        xTc = np.ascontiguousarray(x[b].T).astype(np.float16)
        wq = w_qkv[g * EQ : (g + 1) * EQ]
        wk = w_qkv[D + g * EQ : D + (g + 1) * EQ]
        wv = w_qkv[2 * D + g * EQ : 2 * D + (g + 1) * EQ]
        parts = []
        for e in range(3):
            parts.append(wq[e * P : (e + 1) * P])
            parts.append(wk[e * P : (e + 1) * P])
        wqkTc = np.ascontiguousarray(np.concatenate(parts, 0).T).astype(np.float16)
        wvTc = np.ascontiguousarray(wv.T).astype(np.float16)
        woTc = np.ascontiguousarray(w_o[:, g * EQ : (g + 1) * EQ].T).astype(np.float16)
        in_maps.append(
            {
                "xT": xTc,
                "wqkT": wqkTc,
                "wvT": wvTc,
                "woT": woTc,
                "cos2": cos2,
                "ss2": ss2,
            }
        )
    return in_maps


def kernel(x, w_qkv, w_o, cos, sin):
    nc = _get_nc()
    in_maps = make_in_maps(x, w_qkv, w_o, cos, sin)
    res = run_bass_kernel_spmd(nc, in_maps, core_ids=list(range(8)))
    outs = [res.results[c]["out"].astype(np.float32) for c in range(8)]
    full = np.stack(
        [
            outs[0] + outs[1] + outs[2] + outs[3],
            outs[4] + outs[5] + outs[6] + outs[7],
        ]
    )
    return full
